# revision 1
# baseline (speedup 1.0000x reference)
"""AdaAttModel forward as a distributed Bass kernel on 8 TRN2 NeuronCores.

Sharding: data-parallel over batch (B=64 -> 8 per core), weights replicated.
No collectives: each core computes logp for its batch shard end to end.

Per-core layout conventions (R=512, T=20 steps, A=196, V=7800, F=2048, BL=8):
  - "T-layout" (d on partitions): X^T stored as [128, DC, cols] where
    row p, chunk k holds feature d = k*128+p.
  - (b,t) pairs are flattened as tb = t*8 + b (t-major) so one scan step t
    is a contiguous column slice.
  - gates fold: [128, 24, 8]; col (nc, b) holds gate-feature n = nc*128+p
    for batch b.  Gate g occupies nc = 4g..4g+3, aligning elementwise with
    the state fold [128, 4, 8] (d = dc*128+p).
  - W_h and W_log are fp8(e4m3) scaled x64 on host; descaled on-chip.
"""

import os
import sys

sys.path.insert(0, "/opt/trn_rl_repo")

import numpy as np
import ml_dtypes

import concourse.bass as bass
import concourse.tile as tile
from concourse import mybir, bacc
from concourse import bass_isa
from concourse.bass_utils import run_bass_kernel_spmd

BF = ml_dtypes.bfloat16
FP8 = ml_dtypes.float8_e4m3
F32 = mybir.dt.float32
BF16 = mybir.dt.bfloat16
F8 = mybir.dt.float8e4
AF = mybir.ActivationFunctionType
ALU = mybir.AluOpType
AX = mybir.AxisListType

NCORES = 8
B, TT, A, F, V, R = 64, 21, 196, 2048, 7800, 512
BL = B // NCORES          # 8 local batch
T = TT - 1                # 20 scan steps
NBT = T * BL              # 160
NC6 = 24                  # 6R/128 gate chunks
KC = F // 128             # 16
AJ = 392                  # 1568 = 4*392
NA = BL * A               # 1568
TH = T // 2               # t-half for attention tiles
W8SC = 64.0               # fp8 weight prescale


def build_graph(debug=False):
    nc = bacc.Bacc("TRN2", target_bir_lowering=False, num_devices=NCORES)

    def din(name, shape, dt=BF16):
        return nc.dram_tensor(name, shape, dt, kind="ExternalInput").ap()

    g = {}
    g["attT"] = din("attT", [F, NA])
    g["xgT"] = din("xgT", [R, NBT])
    g["w_ae"] = din("w_ae", [F, R])
    g["w_c2a"] = din("w_c2a", [R, R])
    g["w_word"] = din("w_word", [NC6, 128, 4, 128])
    g["w_h"] = din("w_h", [R, 6 * R], F8)
    g["w_img"] = din("w_img", [NC6, 128, 4, 128])
    g["w_sl"] = din("w_sl", [R, R])
    g["w_se"] = din("w_se", [R, R])
    g["w_hl"] = din("w_hl", [R, R])
    g["w_he"] = din("w_he", [R, R])
    g["w_a2h"] = din("w_a2h", [R, R])
    g["w_al"] = din("w_al", [R, 1])
    g["w_al32"] = din("w_al32", [R, 1], F32)
    g["w_log"] = din("w_log", [R, V], F8)
    g["b_log"] = din("b_log", [1, V])
    for nm in ["b_ae", "b_c2a", "b_sl", "b_se", "b_hl", "b_he", "b_a2h"]:
        g[nm] = din(nm, [128, 4], F32)
    g["b3"] = din("b3", [128, NC6], F32)
    g["out"] = nc.dram_tensor("out", [NBT, V], F32, kind="ExternalOutput").ap()

    g["dbg"] = {}
    if debug:
        for nm, shp in [
            ("d_z", [128, T, NC6, BL]),
            ("d_h", [128, 4, T, BL]),
            ("d_s", [128, 4, T, BL]),
            ("d_scores", [T, 197]),
            ("d_cht", [128, 4, NBT]),
            ("d_afT", [128, 4, NA]),
            ("d_pattT", [128, 4, NA]),
        ]:
            g["dbg"][nm] = nc.dram_tensor(nm, shp, F32, kind="ExternalOutput").ap()

    with tile.TileContext(nc) as tc:
        _body(nc, tc, g, debug)
    nc.compile()
    return nc


def _body(nc, tc, g, debug):
    from contextlib import ExitStack

    ctx = ExitStack()
    dbg = g["dbg"]

    const = ctx.enter_context(tc.tile_pool(name="const", bufs=1))
    live = ctx.enter_context(tc.tile_pool(name="live", bufs=1))
    ps = ctx.enter_context(tc.tile_pool(name="ps", bufs=2, space="PSUM"))

    # -------- whole-kernel constants --------
    def fold_w(pool, key, kc, n, dt=BF16):
        t = pool.tile([128, kc, n], dt, tag=key, name=key + "_sb")
        nc.sync.dma_start(out=t[:], in_=g[key].rearrange("(kc p) n -> p kc n", p=128))
        return t

    w_h = fold_w(const, "w_h", 4, 6 * R, F8)
    w_log = fold_w(const, "w_log", 4, V, F8)
    w_c2a = fold_w(const, "w_c2a", 4, R)
    w_sl = fold_w(const, "w_sl", 4, R)
    w_se = fold_w(const, "w_se", 4, R)
    w_hl = fold_w(const, "w_hl", 4, R)
    w_he = fold_w(const, "w_he", 4, R)
    w_a2h = fold_w(const, "w_a2h", 4, R)
    w_al = fold_w(const, "w_al", 4, 1)
    w_al32 = fold_w(const, "w_al32", 4, 1, F32)
    b_log = const.tile([1, V], BF16, name="b_log_sb")
    nc.sync.dma_start(out=b_log[:], in_=g["b_log"][:, :])

    bias = {}
    for nm in ["b_ae", "b_c2a", "b_sl", "b_se", "b_hl", "b_he", "b_a2h"]:
        bias[nm] = const.tile([128, 4], F32, tag=nm, name=nm + "_sb")
        nc.sync.dma_start(out=bias[nm][:], in_=g[nm][:, :])
    b3 = const.tile([128, NC6], F32, name="b3_sb")
    nc.sync.dma_start(out=b3[:], in_=g["b3"][:, :])

    ident = const.tile([128, 128], BF16, name="ident")
    from concourse.masks import make_identity
    make_identity(nc, ident[:])
    ones1 = const.tile([1, 128], BF16, name="ones1")
    nc.vector.memset(ones1[:], 1.0)

    # -------- long-lived activations --------
    Z = live.tile([128, T, NC6, BL], BF16, name="Z")
    afT = live.tile([128, 4, NA], BF16, name="afT")
    pattT = live.tile([128, 4, NA], BF16, name="pattT")
    af_nat = live.tile([128, BL, 2, R], BF16, name="af_nat")
    Hall = live.tile([128, 4, T, BL], BF16, name="Hall")
    Sall = live.tile([128, 4, T, BL], BF16, name="Sall")

    # ================= phase A =================
    with tc.tile_pool(name="pa", bufs=1) as pa, \
         tc.tile_pool(name="paw", bufs=5) as paw, \
         tc.tile_pool(name="psa", bufs=1, space="PSUM") as psa:
        # A1: wg -> Z
        xg = pa.tile([128, 4, NBT], BF16, name="xg")
        nc.sync.dma_start(out=xg[:], in_=g["xgT"].rearrange("(kc p) m -> p kc m", p=128))
        xrelu = pa.tile([128, 4, NBT], BF16, name="xrelu")
        nc.scalar.activation(xrelu[:], xg[:], AF.Relu)
        for ncg in range(NC6):
            wwk = paw.tile([128, 4, 128], BF16, tag="wwk", name="wwk")
            nc.sync.dma_start(out=wwk[:], in_=g["w_word"][ncg])
            pp = psa.tile([128, NBT], F32, tag="pp_af0", name="pp_wg")
            for k in range(4):
                nc.tensor.matmul(pp[:], wwk[:, k, :], xrelu[:, k, :],
                                 start=(k == 0), stop=(k == 3))
            nc.vector.tensor_copy(Z[:, :, ncg, :], pp[:].rearrange("p (t b) -> p t b", b=BL))

        # A2: afT
        for jg in range(2):
            for ng in range(2):
                pats = [psa.tile([128, AJ], F32, tag=f"pp_af{i}", name=f"pp_af{i}")
                        for i in range(4)]
                for k in range(KC):
                    attk = paw.tile([128, NA // 2], BF16, tag="attk", name="attk")
                    nc.sync.dma_start(
                        out=attk[:],
                        in_=g["attT"][k * 128:(k + 1) * 128, jg * (NA // 2):(jg + 1) * (NA // 2)])
                    waek = paw.tile([128, R], BF16, tag="waek", name="waek")
                    nc.sync.dma_start(out=waek[:], in_=g["w_ae"][k * 128:(k + 1) * 128, :])
                    for nn in range(2):
                        n = ng * 2 + nn
                        for jj in range(2):
                            nc.tensor.matmul(pats[nn * 2 + jj][:],
                                             waek[:, n * 128:(n + 1) * 128],
                                             attk[:, jj * AJ:(jj + 1) * AJ],
                                             start=(k == 0), stop=(k == KC - 1))
                for nn in range(2):
                    n = ng * 2 + nn
                    for jj in range(2):
                        j = jg * 2 + jj
                        nc.scalar.activation(afT[:, n, j * AJ:(j + 1) * AJ],
                                             pats[nn * 2 + jj][:], AF.Relu,
                                             bias=bias["b_ae"][:, n:n + 1])
        if debug:
            _dump(nc, pa, dbg["d_afT"], afT[:])

        # A3: p_attT
        for n in range(4):
            for j in range(4):
                pp = psa.tile([128, AJ], F32, tag="pp_af2", name="pp_patt")
                for k in range(4):
                    nc.tensor.matmul(pp[:], w_c2a[:, k, n * 128:(n + 1) * 128],
                                     afT[:, k, j * AJ:(j + 1) * AJ],
                                     start=(k == 0), stop=(k == 3))
                nc.scalar.activation(pattT[:, n, j * AJ:(j + 1) * AJ], pp[:],
                                     AF.Identity, bias=bias["b_c2a"][:, n:n + 1])
        if debug:
            _dump(nc, pa, dbg["d_pattT"], pattT[:])

        # A4: af natural via PE transpose
        for b in range(BL):
            for n in range(4):
                for ac in range(2):
                    asz = 128 if ac == 0 else A - 128
                    pt = psa.tile([128, 128], BF16, tag="pp_af3", name="pp_tr")
                    nc.tensor.transpose(pt[0:asz, 0:128],
                                        afT[:, n, b * A + ac * 128: b * A + ac * 128 + asz],
                                        ident[:, :])
                    nc.vector.tensor_copy(af_nat[0:asz, b, ac, n * 128:(n + 1) * 128],
                                          pt[0:asz, 0:128])

        # A5: mean, img gates, Z += zbase
        afmean = pa.tile([128, 4, BL], F32, name="afmean")
        for n in range(4):
            nc.vector.tensor_reduce(afmean[:, n, :],
                                    afT[:, n, :].rearrange("p (b a) -> p b a", a=A),
                                    AX.X, ALU.add)
        afmean_bf = pa.tile([128, 4, BL], BF16, name="afmean_bf")
        nc.vector.tensor_copy(afmean_bf[:], afmean[:])
        zb_ps = psa.tile([128, NC6, BL], F32, tag="pp_af1", name="pp_ig")
        for ncg in range(NC6):
            wik = paw.tile([128, 4, 128], BF16, tag="wwk", name="wik")
            nc.sync.dma_start(out=wik[:], in_=g["w_img"][ncg])
            for k in range(4):
                nc.tensor.matmul(zb_ps[:, ncg, :], wik[:, k, :],
                                 afmean_bf[:, k, :], start=(k == 0), stop=(k == 3))
        zbase = pa.tile([128, NC6, BL], F32, name="zbase")
        nc.vector.scalar_tensor_tensor(
            out=zbase[:], in0=zb_ps[:], scalar=1.0 / A,
            in1=b3[:, :, None].to_broadcast((128, NC6, BL)),
            op0=ALU.mult, op1=ALU.add)
        nc.vector.tensor_tensor(Z[:], Z[:],
                                zbase[:, None, :, :].to_broadcast((128, T, NC6, BL)),
                                ALU.add)
        if debug:
            _dump(nc, pa, dbg["d_z"], Z[:])

    # ================= phase B+C fused: scan with per-block attention =================
    BLK = 5
    with tc.tile_pool(name="pb", bufs=4) as pb, \
         tc.tile_pool(name="pc", bufs=1) as pc, \
         tc.tile_pool(name="patt", bufs=2) as patt, \
         tc.tile_pool(name="psml", bufs=2) as psml:

        sentlin = pc.tile([128, 4, NBT], BF16, tag="sentlin", name="sentlin")
        sentemb = pc.tile([128, 4, NBT], BF16, tag="sentemb", name="sentemb")
        hlin = pc.tile([128, 4, NBT], BF16, tag="hlin", name="hlin")
        hemb = pc.tile([128, 4, NBT], BF16, tag="hemb", name="hemb")
        scN = [pc.tile([T, 197], F32, tag=f"scN{b}", name=f"scN{b}")
               for b in range(BL)]

        h0 = const.tile([128, 4, BL], BF16, name="h0")
        nc.vector.memset(h0[:], 0.0)
        c_prev = pb.tile([128, 4, BL], F32, tag="c", name="c_init")
        nc.vector.memset(c_prev[:], 0.0)

        with tc.tile_pool(name="psb", bufs=1, space="PSUM") as psb:
            def proj_blk(w, b_ap, rhs_cols, dest, act, cols, nm):
                ncols = cols.stop - cols.start
                for n in range(4):
                    pp = psb.tile([128, ncols], F32, tag="pmm", bufs=1, name="pp_" + nm,
                                  padded_shape=[128, 10 * BL])
                    for k in range(4):
                        nc.tensor.matmul(pp[:], w[:, k, n * 128:(n + 1) * 128],
                                         rhs_cols[k], start=(k == 0), stop=(k == 3))
                    nc.scalar.activation(dest[:, n, cols], pp[:], act, bias=b_ap[:, n:n + 1])

            for t in range(T):
                hprev = h0[:, :, :] if t == 0 else Hall[:, :, t - 1, :]
                gp = ps.tile([128, NC6, BL], F32, tag="mm", name="pp_gates")
                for ncg in range(NC6):
                    for k in range(4):
                        nc.tensor.matmul(gp[:, ncg, :], w_h[:, k, ncg * 128:(ncg + 1) * 128],
                                         hprev[:, k, :], start=(k == 0), stop=(k == 3))
                gs = pb.tile([128, NC6, BL], F32, tag="gs", name="gs")
                nc.vector.scalar_tensor_tensor(out=gs[:], in0=gp[:], scalar=1.0 / W8SC,
                                               in1=Z[:, t, :, :], op0=ALU.mult, op1=ALU.add)
                sig = pb.tile([128, 16, BL], F32, tag="sig", name="sig")
                nc.scalar.activation(sig[:], gs[:, 0:16, :], AF.Sigmoid)
                cell = pb.tile([128, 4, BL], F32, tag="cell", name="cell")
                nc.vector.tensor_tensor(cell[:], gs[:, 16:20, :], gs[:, 20:24, :], ALU.max)
                fc = pb.tile([128, 4, BL], F32, tag="fc", name="fc")
                nc.vector.tensor_tensor(fc[:], sig[:, 4:8, :], c_prev[:], ALU.mult)
                ic = pb.tile([128, 4, BL], F32, tag="ic", name="ic")
                nc.vector.tensor_tensor(ic[:], sig[:, 0:4, :], cell[:], ALU.mult)
                nc.vector.tensor_tensor(fc[:], fc[:], ic[:], ALU.add)
                c_new = pb.tile([128, 4, BL], F32, tag="c", name="c_new")
                nc.scalar.activation(c_new[:], fc[:], AF.Tanh)
                nc.vector.tensor_tensor(Sall[:, :, t, :], sig[:, 12:16, :], c_new[:], ALU.mult)
                nc.vector.tensor_tensor(Hall[:, :, t, :], sig[:, 8:12, :], c_new[:], ALU.mult)
                c_prev = c_new

                if t % 10 == 9:
                    tp = t - 9
                    cols = slice(tp * BL, (t + 1) * BL)
                    proj_blk(w_sl, bias["b_sl"],
                             [Sall[:, k, tp:t + 1, :].rearrange("p t b -> p (t b)")
                              for k in range(4)],
                             sentlin, AF.Relu, cols, "sl")
                    proj_blk(w_hl, bias["b_hl"],
                             [Hall[:, k, tp:t + 1, :].rearrange("p t b -> p (t b)")
                              for k in range(4)],
                             hlin, AF.Tanh, cols, "hl")
                    proj_blk(w_se, bias["b_se"],
                             [sentlin[:, k, cols] for k in range(4)],
                             sentemb, AF.Identity, cols, "se")
                    proj_blk(w_he, bias["b_he"],
                             [hlin[:, k, cols] for k in range(4)],
                             hemb, AF.Identity, cols, "he")
                    he_v = hemb[:].rearrange("p k (t b) -> p k t b", b=BL)
                    se_v = sentemb[:].rearrange("p k (t b) -> p k t b", b=BL)
                    for sub in range((t - tp + 1) // BLK):
                        t0 = tp + sub * BLK
                        t1 = t0 + BLK - 1
                        for b in range(BL):
                            hAtB = patt.tile([128, 4, BLK, 197], BF16, tag="ba", name="hAtB")
                            for n in range(4):
                                nc.vector.tensor_tensor(hAtB[:, n, :, 0],
                                                        se_v[:, n, t0:t1 + 1, b],
                                                        he_v[:, n, t0:t1 + 1, b], ALU.add)
                                nc.vector.tensor_tensor(
                                    hAtB[:, n, :, 1:],
                                    pattT[:, n, None, b * A:(b + 1) * A].to_broadcast((128, BLK, A)),
                                    he_v[:, n, t0:t1 + 1, b][:, :, None].to_broadcast((128, BLK, A)),
                                    ALU.add)
                                nc.scalar.activation(
                                    hAtB[:, n, :, :].rearrange("p t a -> p (t a)"),
                                    hAtB[:, n, :, :].rearrange("p t a -> p (t a)"), AF.Tanh)
                            sp = psb.tile([1, BLK * 197], F32, tag="spsum", bufs=2, name="spsum")
                            for j in range(2):
                                j0 = j * 512
                                js = 512 if j == 0 else BLK * 197 - 512
                                for n in range(4):
                                    nc.tensor.matmul(
                                        sp[:, j0:j0 + js], w_al[:, n, :],
                                        hAtB[:, n, :, :].rearrange("p t a -> p (t a)")[:, j0:j0 + js],
                                        start=(n == 0), stop=(n == 3))
                            s1b = psml.tile([1, BLK * 197], F32, tag="s1b", bufs=2, name="s1b")
                            nc.vector.tensor_copy(s1b[:], sp[:])
                            nc.sync.dma_start(out=scN[b][t0:t1 + 1, :], in_=s1b[:])

            if debug:
                _dump(nc, const, dbg["d_h"], Hall[:])
                _dump(nc, const, dbg["d_s"], Sall[:])

        with tc.tile_pool(name="psc", bufs=2, space="PSUM") as psc:
            # ---------- post-scan: softmax + cHat per b ----------
            chT = pc.tile([128, 4, NBT], BF16, name="chT")
            for b in range(BL):
                if debug and b == 0:
                    _dump(nc, pc, dbg["d_scores"], scN[0][:])
                nmax = psml.tile([T, 1], F32, tag="nmax", name="nmax")
                nc.vector.tensor_reduce(nmax[:], scN[b][:], AX.X, ALU.max, negate=True)
                expn = psml.tile([T, 197], F32, tag="expn", name="expn")
                sume = psml.tile([T, 1], F32, tag="sume", name="sume")
                nc.scalar.activation(expn[:], scN[b][:], AF.Exp, bias=nmax[:], accum_out=sume[:])
                rse = psml.tile([T, 1], F32, tag="rse", name="rse")
                nc.vector.reciprocal(rse[:], sume[:])
                al_nat = psml.tile([T, 197], BF16, tag="al_nat", name="al_nat")
                nc.vector.tensor_scalar(out=al_nat[:], in0=expn[:], scalar1=rse[:],
                                        scalar2=None, op0=ALU.mult)
                a_t0 = psml.tile([128, T], BF16, tag="a_t0", name="a_t0")
                pt = psc.tile([128, T], BF16, tag="tr", name="pp_tra")
                nc.tensor.transpose(pt[0:128, :], al_nat[:, 1:129], ident[0:T, 0:T])
                nc.vector.tensor_copy(a_t0[:, :], pt[0:128, :])
                a_t1 = psml.tile([128, T], BF16, tag="a_t1", name="a_t1")
                pt2 = psc.tile([128, T], BF16, tag="tr", name="pp_trb")
                nc.tensor.transpose(pt2[0:A - 128, :], al_nat[:, 129:197], ident[0:T, 0:T])
                nc.vector.tensor_copy(a_t1[0:A - 128, :], pt2[0:A - 128, :])
                a0r = psml.tile([1, T], BF16, tag="a0r", name="a0r")
                pt3 = psc.tile([128, T], BF16, tag="tr", name="pp_trc")
                nc.tensor.transpose(pt3[0:1, :], al_nat[:, 0:1], ident[0:T, 0:T])
                nc.vector.tensor_copy(a0r[:], pt3[0:1, :])
                pa0 = psc.tile([128, T], F32, tag="mm3", name="pp_a0")
                nc.tensor.matmul(pa0[:], ones1[0:1, :], a0r[0:1, :], start=True, stop=True)
                cp = psc.tile([128, 4, T], F32, tag="mm2", name="pp_chat")
                for dc in range(4):
                    nc.tensor.matmul(cp[:, dc, :], af_nat[:, b, 0, dc * 128:(dc + 1) * 128],
                                     a_t0[:, :], start=True, stop=False)
                    nc.tensor.matmul(cp[:, dc, :],
                                     af_nat[0:A - 128, b, 1, dc * 128:(dc + 1) * 128],
                                     a_t1[0:A - 128, :], start=False, stop=True)
                sl_b = sentlin[:].rearrange("p k (t b) -> p k t b", b=BL)
                hl_b = hlin[:].rearrange("p k (t b) -> p k t b", b=BL)
                for dc in range(4):
                    tmp = psml.tile([128, T], F32, tag="tmp_sent", name="tmp_sent")
                    nc.vector.tensor_tensor(tmp[:], pa0[:], sl_b[:, dc, :, b], ALU.mult)
                    nc.vector.tensor_tensor(tmp[:], tmp[:], cp[:, dc, :], ALU.add)
                    nc.vector.tensor_tensor(chT[:, dc, b::BL], tmp[:], hl_b[:, dc, :, b],
                                            ALU.add)
            if debug:
                _dump(nc, pc, dbg["d_cht"], chT[:])

        outT = pc.tile([128, 4, NBT], BF16, name="outT")
        for n in range(4):
            pp = ps.tile([128, NBT], F32, tag="mm", name="pp_a2h")
            for k in range(4):
                nc.tensor.matmul(pp[:], w_a2h[:, k, n * 128:(n + 1) * 128],
                                 chT[:, k, :], start=(k == 0), stop=(k == 3))
            nc.scalar.activation(outT[:, n, :], pp[:], AF.Tanh, bias=bias["b_a2h"][:, n:n + 1])

        # ---------- phase D: logits + log_softmax ----------
        NVT = 16
        for mc in range(2):
            ms = 128 if mc == 0 else NBT - 128
            lgb = patt.tile([128, V], BF16, tag="ba", name="lgb")
            tmaxs = psml.tile([128, NVT], F32, tag="tmaxs", name="tmaxs")
            for vt in range(NVT):
                vs = 512 if vt < NVT - 1 else V - 512 * (NVT - 1)
                lp = ps.tile([128, 512], F32, tag="mm", name="pp_log")
                for k in range(4):
                    nc.tensor.matmul(lp[0:ms, 0:vs],
                                     outT[:, k, mc * 128:mc * 128 + ms],
                                     w_log[:, k, vt * 512:vt * 512 + vs],
                                     start=(k == 0), stop=False)
                nc.tensor.matmul(lp[0:ms, 0:vs], ones1[0:1, 0:ms],
                                 b_log[0:1, vt * 512:vt * 512 + vs],
                                 start=False, stop=True)
                nc.vector.tensor_reduce(tmaxs[0:ms, vt:vt + 1], lp[0:ms, 0:vs],
                                        AX.X, ALU.max)
                # descale the x64 fp8 weight prescale on the way out of PSUM
                nc.vector.tensor_scalar(out=lgb[0:ms, vt * 512:vt * 512 + vs],
                                        in0=lp[0:ms, 0:vs], scalar1=1.0 / W8SC,
                                        scalar2=None, op0=ALU.mult)
            nmaxL = psml.tile([128, 1], F32, tag="nmaxL", name="nmaxL")
            nc.vector.tensor_reduce(nmaxL[0:ms, :], tmaxs[0:ms, :], AX.X, ALU.max,
                                    negate=True)
            nc.vector.tensor_scalar(out=nmaxL[0:ms, :], in0=nmaxL[0:ms, :],
                                    scalar1=1.0 / W8SC, scalar2=None, op0=ALU.mult)
            expb = patt.tile([128, V], BF16, tag="ba", name="expb")
            sumL = psml.tile([128, 1], F32, tag="sumL", name="sumL")
            nc.scalar.activation(expb[0:ms, :], lgb[0:ms, :], AF.Exp,
                                 bias=nmaxL[0:ms, :], accum_out=sumL[0:ms, :])
            lnL = psml.tile([128, 1], F32, tag="lnL", name="lnL")
            nc.scalar.activation(lnL[0:ms, :], sumL[0:ms, :], AF.Ln)
            shf = psml.tile([128, 1], F32, tag="shf", name="shf")
            nc.vector.tensor_tensor(shf[0:ms, :], nmaxL[0:ms, :], lnL[0:ms, :],
                                    ALU.subtract)
            for vt in range(NVT):
                vs = 512 if vt < NVT - 1 else V - 512 * (NVT - 1)
                fo = psml.tile([128, 512], F32, tag="fo", bufs=2, name="fo")
                nc.vector.tensor_scalar(out=fo[0:ms, 0:vs],
                                        in0=lgb[0:ms, vt * 512:vt * 512 + vs],
                                        scalar1=shf[0:ms, :], scalar2=None, op0=ALU.add)
                nc.sync.dma_start(
                    out=g["out"][mc * 128:mc * 128 + ms, vt * 512:vt * 512 + vs],
                    in_=fo[0:ms, 0:vs])
    ctx.close()


def _dump(nc, pool, dram_ap, src_ap):
    shp = list(src_ap.shape)
    t = pool.tile(shp, F32, tag="dbgc", name="dbgc")
    nc.vector.tensor_copy(t[:], src_ap)
    nc.sync.dma_start(out=dram_ap, in_=t[:])


# ------------------------------------------------------------------
_CACHE = {}


def _get_graph():
    if "nc" not in _CACHE:
        _CACHE["nc"] = build_graph(debug=bool(int(os.environ.get("KDBG", "0"))))
    return _CACHE["nc"]


def prep_inputs(inputs):
    seq = np.asarray(inputs["seq"])
    att = np.asarray(inputs["att_feats"], dtype=np.float32)
    embed_w = np.asarray(inputs["embed_w"], dtype=np.float32)

    def bf(x):
        return np.ascontiguousarray(np.asarray(x, dtype=np.float32)).astype(BF)

    def f8(x):
        return np.ascontiguousarray(np.asarray(x, np.float32) * W8SC).astype(FP8)

    def fold_bias(v, nch):
        return np.ascontiguousarray(np.asarray(v, np.float32).reshape(nch, 128).T)

    shared = {
        "w_ae": bf(inputs["W_ae"]), "w_c2a": bf(inputs["W_c2a"]),
        "w_word": bf(np.asarray(inputs["W_word"], np.float32).reshape(4, 128, NC6, 128).transpose(2, 1, 0, 3)), "w_h": f8(inputs["W_h"]),
        "w_img": bf(np.asarray(inputs["W_img"], np.float32).reshape(4, 128, NC6, 128).transpose(2, 1, 0, 3)), "w_sl": bf(inputs["W_sl"]),
        "w_se": bf(inputs["W_se"]), "w_hl": bf(inputs["W_hl"]),
        "w_he": bf(inputs["W_he"]), "w_a2h": bf(inputs["W_a2h"]),
        "w_al": bf(inputs["W_al"]),
        "w_al32": np.ascontiguousarray(np.asarray(inputs["W_al"], np.float32)), "w_log": f8(inputs["W_log"]),
        "b_log": bf(np.asarray(inputs["b_log"], np.float32)[None, :] * W8SC),
        "b_ae": fold_bias(inputs["b_ae"], 4), "b_c2a": fold_bias(inputs["b_c2a"], 4),
        "b_sl": fold_bias(inputs["b_sl"], 4), "b_se": fold_bias(inputs["b_se"], 4),
        "b_hl": fold_bias(inputs["b_hl"], 4), "b_he": fold_bias(inputs["b_he"], 4),
        "b_a2h": fold_bias(inputs["b_a2h"], 4),
        "b3": fold_bias(np.asarray(inputs["b_word"], np.float32)
                        + np.asarray(inputs["b_h"], np.float32)
                        + np.asarray(inputs["b_img"], np.float32), NC6),
    }
    in_maps = []
    for c in range(NCORES):
        sl = slice(c * BL, (c + 1) * BL)
        attT = np.ascontiguousarray(att[sl].reshape(BL * A, F).T).astype(BF)
        gth = embed_w[seq[sl, :T]]                      # [BL, T, R]
        xgT = np.ascontiguousarray(gth.transpose(2, 1, 0).reshape(R, T * BL)).astype(BF)
        m = dict(shared)
        m["attT"] = attT
        m["xgT"] = xgT
        in_maps.append(m)
    return in_maps


def kernel(**inputs):
    in_maps = prep_inputs(inputs)
    nc = _get_graph()
    res = run_bass_kernel_spmd(nc, in_maps, core_ids=list(range(NCORES)),
                               **_CACHE.get("run_kwargs", {}))
    outs = []
    for c in range(NCORES):
        o = np.asarray(res.results[c]["out"])           # [160, 7800], rows tb
        outs.append(o.reshape(T, BL, V).transpose(1, 0, 2))
    full = np.concatenate(outs, axis=0).astype(np.float32)
    _CACHE["last_results"] = res
    return full


if __name__ == "__main__":
    build_graph(debug=bool(int(os.environ.get("KDBG", "0"))))
    print("graph built OK")

